# revision 28
# baseline (speedup 1.0000x reference)
"""Causal multi-head attention (B=2, S=2048, D=1024, H=16) on 8 trn2 cores.

Sharding v2: core c handles batch b = c//4 and heads {4r..4r+3} (r = c%4),
i.e. cores 0-3 cover batch 0, cores 4-7 batch 1.  Per core:

  - project the host-pretransposed x_b^T [D, S] (OWN batch only) through the
    core's Wqkv column slice into Q^T/K^T head-pair tiles (fp16) and V in
    natural layout with a fused ones-column (so the AV matmul also emits the
    softmax denominators),
  - causal attention per (head-pair, qblock) in transposed layout, fp16
    operands with fp32 PSUM accumulation: scores^T = K Q^T (row-tiled head
    pairs), exp on ScalarE, diagonal masks on GpSimd, A^T V on PE,
  - after each 512-query block, a 4-wide AllToAll (replica groups
    [0-3],[4-7]) redistributes that quarter's head outputs so core (b,r)
    receives ALL 16 heads for queries 512*q + 128*r .. +128; these four
    collectives overlap with the remaining attention compute,
  - the output projection through the full Wout (fp16) runs per received
    128-query chunk, pipelined behind the collectives.

Host assembles the 8 cores x 4 chunks of [128, 1024] into (2, 2048, 1024).

Projection matmuls run in float32r (TF32-like, ~1e-3 rel err); everything
downstream of the projections is fp16 (≥10-bit mantissa, same PE throughput,
half the SBUF/DMA/collective bytes).
"""

import sys

for _p in ("/opt/trn_rl_repo", "/opt/pypackages"):
    if _p not in sys.path:
        sys.path.insert(0, _p)

import numpy as np

import concourse.bass as bass
import concourse.mybir as mybir
import concourse.tile as tile
from concourse import bacc
from concourse.bass_utils import run_bass_kernel_spmd

B = 2
S = 2048
D = 1024
H = 16
DH = 64
NCORES = 8
SB = 512           # q block (matmul moving dim)
KC = 128           # k chunk (contraction tile)
NSB = S // SB      # 4 q-blocks
NKC = S // KC      # 16 k-chunks
NDC = D // KC      # 8 contraction chunks for the projections

_compiled = None


def _build():
    f32 = mybir.dt.float32
    f16 = mybir.dt.float16
    fr = mybir.dt.float32r
    nc = bacc.Bacc(None, target_bir_lowering=False)

    # host-blocked inputs (own batch / own 4 heads only), fp16
    xt = nc.declare_dram_parameter("xt", [NSB, NDC, KC, SB], f16, isOutput=False)
    wqk = nc.declare_dram_parameter("wqk", [NDC, KC, 4 * KC], f16, isOutput=False)
    wv = nc.declare_dram_parameter("wv", [NDC, KC, 2 * KC], f16, isOutput=False)
    wout = nc.declare_dram_parameter("wout", [2, NDC, KC, SB], f16, isOutput=False)
    bqk = nc.declare_dram_parameter("bqk", [KC, 4], f32, isOutput=False)
    bv = nc.declare_dram_parameter("bv", [1, 2 * KC], f32, isOutput=False)
    bo = nc.declare_dram_parameter("bo", [1, D], f32, isOutput=False)
    vones = nc.declare_dram_parameter("vones", [KC, 16], f16, isOutput=False)
    out_ext = nc.declare_dram_parameter("out", [NSB, KC, D], f32, isOutput=True)

    # per-(quarter, head-pair) AllToAll staging: a2a_in[q][hp][t] = this
    # core's head-pair hp output (transposed, [128 head dims, 128 queries])
    # for query sub-chunk 512*q + 128*(t%4).  The exchange is logically
    # within each batch group of 4 cores, but the collective stack only
    # supports 8-wide AllToAll (mesh), so both group halves carry the same
    # data and each receiver dynamically slices its own group's 4 sender
    # blocks.  Splitting per head-pair lets each half-collective start as
    # soon as that head-pair's attention quarter is done.
    a2a_in = [[nc.dram_tensor(f"a2a_in{q}_{hp}", [NCORES, KC, KC], f16)
               for hp in range(2)] for q in range(NSB)]
    a2a_out = [[nc.dram_tensor(f"a2a_out{q}_{hp}", [NCORES, KC, KC], f16)
                for hp in range(2)] for q in range(NSB)]
    groups = [[0, 1, 2, 3, 4, 5, 6, 7]]

    with tile.TileContext(nc) as tc:
        with (
            tc.tile_pool(name="misc", bufs=1) as mp,
            tc.tile_pool(name="weights", bufs=1) as wp,
            tc.tile_pool(name="xbuf", bufs=1) as xp,
            tc.tile_pool(name="qkv", bufs=1) as qkvp,
            tc.tile_pool(name="pbuf", bufs=1) as pb,
            tc.tile_pool(name="obuf", bufs=1) as op,
            tc.tile_pool(name="evict", bufs=1) as ep,
            tc.tile_pool(name="psum", bufs=1, space="PSUM") as pp,
        ):
            # ---- PE warmup while the initial DMAs land --------------------
            wdum = mp.tile([KC, KC], f16, tag="wdum")
            nc.vector.memset(wdum[:], 0.0)
            for i in range(64):
                psd = pp.tile([KC, SB], f32, tag="accum", bufs=2)
                nc.tensor.matmul(psd[:, 0:KC], wdum[:], wdum[:],
                                 start=True, stop=True)

            # ---- big loads first: unblock the first projection ASAP -------
            # wqk_t cols: k*512 + m*128, m in {Q01, Q23, K01, K23}; first
            # halves (k 0:4) land first so proj(0) can begin sooner
            wqk_t = wp.tile([KC, NDC * 4 * KC], f16, tag="wqk")
            xts = []
            for s in range(NSB):
                xts.append(xp.tile([KC, NDC * SB], f16, tag=f"xt{s}",
                                   name=f"xt{s}"))
            HD = NDC // 2
            for h in range(2):
                nc.gpsimd.dma_start(
                    out=wqk_t[:, h * HD * 4 * KC:(h + 1) * HD * 4 * KC]
                    .rearrange("p (k c) -> p k c", k=HD),
                    in_=wqk[h * HD:(h + 1) * HD].rearrange("k p c -> p k c"),
                )
                nc.gpsimd.dma_start(
                    out=xts[0][:, h * HD * SB:(h + 1) * HD * SB]
                    .rearrange("p (k c) -> p k c", k=HD),
                    in_=xt[0, h * HD:(h + 1) * HD].rearrange("k p c -> p k c"),
                )
            wv_t = wp.tile([KC, NDC * 2 * KC], f16, tag="wv")
            nc.gpsimd.dma_start(
                out=wv_t[:].rearrange("p (k c) -> p k c", k=NDC),
                in_=wv[:].rearrange("k p c -> p k c"),
            )
            nc.gpsimd.dma_start(
                out=xts[1][:].rearrange("p (k c) -> p k c", k=NDC),
                in_=xt[1].rearrange("k p c -> p k c"),
            )

            # ---- small constants -----------------------------------------
            bqk_t = mp.tile([KC, 4], f32, tag="bqk")
            nc.sync.dma_start(out=bqk_t[:], in_=bqk[:])
            bv_row = mp.tile([1, 2 * KC], f32, tag="bv_row")
            nc.sync.dma_start(out=bv_row[:], in_=bv[:])
            bv_bc = mp.tile([KC, 2 * KC], f32, tag="bv_bc")
            nc.gpsimd.partition_broadcast(out_ap=bv_bc[:], in_ap=bv_row[:])
            bo_row = mp.tile([1, D], f32, tag="bo_row")
            nc.sync.dma_start(out=bo_row[:], in_=bo[:])
            bo_bc = mp.tile([KC, D], f32, tag="bo_bc")
            nc.gpsimd.partition_broadcast(out_ap=bo_bc[:], in_ap=bo_row[:])
            vones_sb = mp.tile([KC, 16], f16, tag="vones_sb")
            nc.sync.dma_start(out=vones_sb[:], in_=vones[:])

            # ---- persistent activations ----------------------------------
            # QQ[hp][s]: rows 0:64 = Q^T head 4r+2hp, 64:128 = head 4r+2hp+1
            QQ = [[qkvp.tile([KC, SB], f16, tag=f"QQ{hp}_{s}",
                             name=f"QQ{hp}_{s}") for s in range(NSB)]
                  for hp in range(2)]
            KK = [[qkvp.tile([KC, SB], f16, tag=f"KK{hp}_{s}",
                             name=f"KK{hp}_{s}") for s in range(NSB)]
                  for hp in range(2)]
            # V_all[s]: [128, 4 heads * 4 sc * 65]; head v block at v*260,
            # chunk sc at v*260 + sc*65, col 64 of each chunk = 1.0
            NCS = SB // KC
            V_all = [qkvp.tile([KC, 4 * NCS * (DH + 1)], f16, tag=f"V{s}",
                               name=f"V{s}") for s in range(NSB)]
            for s in range(NSB):
                vv = V_all[s][:].rearrange("p (v c) -> p v c", c=DH + 1)
                nc.vector.tensor_copy(vv[:, :, DH], vones_sb[:])
            # O[hp]: rows 0:64 = head 4r+2hp out^T (normalized), 64:128 =
            # head 4r+2hp+1
            O = [op.tile([KC, S], f16, tag=f"O{hp}", name=f"O{hp}")
                 for hp in range(2)]
            # wout (fp16, 2MB) loaded whole; needed from first out-proj on
            wout_t = wp.tile([KC, 2 * NDC * SB], f16, tag="wout")

            def proj(s):
                """QKV projection for seq block s (f32r)."""
                xs = xts[s]
                for m in range(4):
                    ps = pp.tile([KC, SB], f32, tag="accum", bufs=2,
                                 name=f"psqk{m}_{s}")
                    for k in range(NDC):
                        nc.tensor.matmul(
                            ps[:],
                            wqk_t[:, k * 4 * KC + m * KC:
                                  k * 4 * KC + (m + 1) * KC],
                            xs[:, k * SB:(k + 1) * SB],
                            start=(k == 0),
                            stop=(k == NDC - 1),
                        )
                        if s == 0 and m == 0 and k == 3:
                            # early wout kick: overlaps with projections
                            nc.gpsimd.dma_start(
                                out=wout_t[:].rearrange(
                                    "p (k c) -> p k c", k=2 * NDC),
                                in_=wout[:].rearrange(
                                    "n k p c -> p (n k) c"),
                            )
                    dest = (QQ if m < 2 else KK)[m % 2][s]
                    nc.vector.tensor_scalar_add(
                        dest[:], ps[:], bqk_t[:, m:m + 1])
                for sc in range(NCS):
                    pv = pp.tile([KC, 2 * KC], f32, tag="accum", bufs=2,
                                 name=f"psv{sc}_{s}")
                    for k in range(NDC):
                        nc.tensor.matmul(
                            pv[:],
                            xs[:, k * SB + sc * KC:k * SB + (sc + 1) * KC],
                            wv_t[:, k * 2 * KC:(k + 1) * 2 * KC],
                            start=(k == 0),
                            stop=(k == NDC - 1),
                        )
                    vv = V_all[s][:].rearrange(
                        "p (v k c) -> p v k c", v=4, k=NCS)
                    nc.vector.tensor_add(
                        vv[:, :, sc, 0:DH],
                        pv[:].rearrange("p (v c) -> p v c", c=DH),
                        bv_bc[:].rearrange("p (v c) -> p v c", c=DH),
                    )

            def attn(q, hp):
                """Attention for query block q, one head pair."""
                nkc = 4 * (q + 1)
                if True:
                    P = []
                    for kc in range(nkc):
                        d = kc - 4 * q
                        c0 = KC * max(d, 0)
                        ps = pp.tile([KC, 2 * SB], f32, tag="pss", bufs=2,
                                     name=f"pss{hp}_{q}_{kc}")
                        for hh in range(2):
                            r0 = hh * DH
                            nc.tensor.matmul(
                                ps[:, hh * SB + c0:(hh + 1) * SB],
                                KK[hp][kc // 4][r0:r0 + DH,
                                                (kc % 4) * KC:
                                                (kc % 4 + 1) * KC],
                                QQ[hp][q][r0:r0 + DH, c0:SB],
                                start=True,
                                stop=True,
                            )
                        pt = pb.tile([KC, 2 * SB], f16, tag="P", bufs=8,
                                     name=f"P{hp}_{q}_{kc}")
                        P.append(pt)
                        ps3 = ps[:].rearrange("p (h f) -> p h f", h=2)
                        pd3 = pt[:].rearrange("p (h f) -> p h f", h=2)
                        nc.scalar.activation(
                            pd3[:, :, c0:SB],
                            ps3[:, :, c0:SB],
                            mybir.ActivationFunctionType.Exp,
                            scale=1.0 / float(np.sqrt(DH)),
                        )
                        if d >= 0:  # diagonal chunk: zero where k > q
                            nc.gpsimd.affine_select(
                                out=pd3[:, :, c0:SB],
                                in_=pd3[:, :, c0:SB],
                                pattern=[[0, 2], [1, SB - c0]],
                                compare_op=mybir.AluOpType.is_ge,
                                fill=0.0,
                                base=0,
                                channel_multiplier=-1,
                            )
                    pos = [pp.tile([DH + 1, SB], f32, tag="pos", bufs=2,
                                   name=f"pos{hh}_{hp}_{q}")
                           for hh in range(2)]
                    for kc in range(nkc):
                        d = kc - 4 * q
                        c0 = KC * max(d, 0)
                        for hh in range(2):
                            v = 2 * hp + hh
                            nc.tensor.matmul(
                                pos[hh][:, c0:SB],
                                V_all[kc // 4][:, v * NCS * (DH + 1) +
                                               (kc % 4) * (DH + 1):
                                               v * NCS * (DH + 1) +
                                               (kc % 4 + 1) * (DH + 1)],
                                P[kc][:, hh * SB + c0:(hh + 1) * SB],
                                start=(kc == 0),
                                stop=(kc == nkc - 1),
                            )
                    # normalize: O = pos[0:64] * (1 / pos[64]) per head
                    for hh in range(2):
                        den0 = ep.tile([1, SB], f32, tag=f"den{hh}", bufs=2,
                                       name=f"den{hh}_{hp}_{q}")
                        nc.vector.tensor_copy(den0[:], pos[hh][DH:DH + 1, :])
                        rden = ep.tile([1, SB], f32, tag=f"rden{hh}", bufs=2)
                        rscr = ep.tile([1, SB], f32, tag=f"rscr{hh}", bufs=2)
                        nc.vector.reciprocal_approx_accurate(
                            rden[:], den0[:], rscr[:])
                        rbc = ep.tile([DH, SB], f32, tag=f"rbc{hh}", bufs=2,
                                      name=f"rbc{hh}_{hp}_{q}")
                        nc.gpsimd.partition_broadcast(
                            out_ap=rbc[:],
                            in_ap=rden[:],
                        )
                        r0 = hh * DH
                        nc.vector.tensor_mul(
                            O[hp][r0:r0 + DH, q * SB:(q + 1) * SB],
                            pos[hh][0:DH, :],
                            rbc[:],
                        )
                    # stage this head-pair's quarter (both group halves get
                    # a copy) and fire its half-collective immediately
                    for g in range(2):
                        nc.gpsimd.dma_start(
                            out=a2a_in[q][hp][4 * g:4 * (g + 1)].rearrange(
                                "t p c -> p t c"),
                            in_=O[hp][:, q * SB:(q + 1) * SB].rearrange(
                                "p (t c) -> p t c", t=4),
                        )
                    nc.gpsimd.collective_compute(
                        "AllToAll",
                        mybir.AluOpType.bypass,
                        replica_groups=groups,
                        ins=[a2a_in[q][hp][:]],
                        outs=[a2a_out[q][hp][:]],
                    )

            # my group's sender blocks start at slot 4*(rank//4)
            pid = nc.sync.partition_id()
            soff = (pid // 4) * 4

            po_live = {}

            def outproj(q, hp):
                """Half the output projection for quarter q: contraction
                chunks belonging to head-pair hp (recv cols k = 2*s + hp)."""
                recv = po_live.setdefault(
                    ("recv", q),
                    ep.tile([KC, NDC * KC], f16, tag="recv", bufs=2,
                            name=f"recv{q}"))
                nc.sync.dma_start(
                    out=recv[:].rearrange("p (s h c) -> p s h c",
                                          h=2, c=KC)[:, :, hp],
                    in_=a2a_out[q][hp][bass.ds(soff, 4)].rearrange(
                        "s p c -> p s c"),
                )
                for nb in range(2):
                    if hp == 0:
                        po_live[(q, nb)] = pp.tile(
                            [KC, SB], f32, tag="accum", bufs=2,
                            name=f"po{nb}_{q}")
                    po = po_live[(q, nb)]
                    for s in range(4):
                        k = 2 * s + hp
                        nc.tensor.matmul(
                            po[:],
                            recv[:, k * KC:(k + 1) * KC],
                            wout_t[:, (nb * NDC + k) * SB:
                                   (nb * NDC + k + 1) * SB],
                            start=(k == 0),
                            stop=(k == NDC - 1),
                        )
                    if hp == 1:
                        ot = ep.tile([KC, SB], f32, tag="osb", bufs=2,
                                     name=f"ot{nb}_{q}")
                        nc.vector.tensor_add(
                            ot[:], po[:], bo_bc[:, nb * SB:(nb + 1) * SB])
                        nc.sync.dma_start(
                            out=out_ext[q, :, nb * SB:(nb + 1) * SB],
                            in_=ot[:],
                        )

            # ---- pipeline -------------------------------------------------
            proj(0)
            nc.gpsimd.dma_start(
                out=xts[2][:].rearrange("p (k c) -> p k c", k=NDC),
                in_=xt[2].rearrange("k p c -> p k c"),
            )
            proj(1)
            nc.gpsimd.dma_start(
                out=xts[3][:].rearrange("p (k c) -> p k c", k=NDC),
                in_=xt[3].rearrange("k p c -> p k c"),
            )
            attn(0, 0)
            attn(0, 1)
            proj(2)
            attn(1, 0)
            attn(1, 1)
            proj(3)
            outproj(0, 0)
            outproj(0, 1)
            attn(2, 0)
            attn(2, 1)
            outproj(1, 0)
            outproj(1, 1)
            attn(3, 0)
            outproj(2, 0)
            outproj(2, 1)
            attn(3, 1)
            outproj(3, 0)
            outproj(3, 1)

    nc.compile()
    return nc


def _get_program():
    global _compiled
    if _compiled is None:
        _compiled = _build()
    return _compiled


def _shard_inputs(x, Wqkv, bqkv, Wout, bout):
    """Build the 8 per-core input maps (all host-side numpy)."""
    x = np.ascontiguousarray(x, dtype=np.float32)
    Wqkv = np.asarray(Wqkv, dtype=np.float32)
    bqkv = np.asarray(bqkv, dtype=np.float32)
    Wout = np.asarray(Wout, dtype=np.float32)
    bout = np.ascontiguousarray(np.asarray(bout, dtype=np.float32))

    Wq = Wqkv[:, 0 * D:1 * D]
    Wk = Wqkv[:, 1 * D:2 * D]
    Wv_full = Wqkv[:, 2 * D:3 * D]
    bq = bqkv[0 * D:1 * D]
    bk = bqkv[1 * D:2 * D]
    bv_full = bqkv[2 * D:3 * D]

    # per batch: [NSB, NDC, KC, SB] blocked transpose of x (fp16)
    xts = []
    for b in range(B):
        xts.append(np.ascontiguousarray(
            x[b].T                                   # [D, S]
            .reshape(NDC, KC, NSB, SB).transpose(2, 0, 1, 3)
            .astype(np.float16)
        ))
    wout_b = np.ascontiguousarray(
        Wout.reshape(NDC, KC, 2, SB).transpose(2, 0, 1, 3)
        .astype(np.float16))
    bo_row = np.ascontiguousarray(bout.reshape(1, D))
    vones = np.ones((KC, 16), dtype=np.float16)

    in_maps = []
    for c in range(NCORES):
        b, r = c // 4, c % 4
        h0 = 4 * r
        cols = lambda W, i: W[:, (h0 + i) * DH:(h0 + i + 2) * DH]
        wqk_c = np.ascontiguousarray(np.concatenate(
            [cols(Wq, 0), cols(Wq, 2), cols(Wk, 0), cols(Wk, 2)],
            axis=1).reshape(NDC, KC, 4 * KC).astype(np.float16))
        bqk_c = np.ascontiguousarray(np.stack(
            [bq[(h0) * DH:(h0 + 2) * DH], bq[(h0 + 2) * DH:(h0 + 4) * DH],
             bk[(h0) * DH:(h0 + 2) * DH], bk[(h0 + 2) * DH:(h0 + 4) * DH]],
            axis=1))
        wv_c = np.ascontiguousarray(
            Wv_full[:, h0 * DH:(h0 + 4) * DH].reshape(NDC, KC, 2 * KC)
            .astype(np.float16))
        bv_c = np.ascontiguousarray(
            bv_full[h0 * DH:(h0 + 4) * DH].reshape(1, 2 * KC))
        in_maps.append({
            "xt": xts[b], "wqk": wqk_c, "wv": wv_c, "wout": wout_b,
            "bqk": bqk_c, "bv": bv_c, "bo": bo_row, "vones": vones,
        })
    return in_maps


def run(inputs, trace=False, trace_kwargs=None):
    nc = _get_program()
    in_maps = _shard_inputs(**inputs)
    res = run_bass_kernel_spmd(
        nc, in_maps, list(range(NCORES)), trace=trace,
        **(trace_kwargs or {}),
    )
    out = np.empty((B, S, D), dtype=np.float32)
    for c in range(NCORES):
        b, r = c // 4, c % 4
        o = res.results[c]["out"]          # [NSB, KC, D]
        for q in range(NSB):
            out[b, SB * q + KC * r:SB * q + KC * (r + 1), :] = o[q]
    return out, res


def kernel(**inputs):
    out, _ = run(inputs)
    return out


# revision 32
# speedup vs baseline: 1.2717x; 1.2717x over previous
"""Causal multi-head attention (B=2, S=2048, D=1024, H=16) on 8 trn2 cores.

Sharding v2: core c handles batch b = c//4 and heads {4r..4r+3} (r = c%4),
i.e. cores 0-3 cover batch 0, cores 4-7 batch 1.  Per core:

  - project the host-pretransposed x_b^T [D, S] (OWN batch only) through the
    core's Wqkv column slice into Q^T/K^T head-pair tiles (fp16) and V in
    natural layout with a fused ones-column (so the AV matmul also emits the
    softmax denominators),
  - causal attention per (head-pair, qblock) in transposed layout, fp16
    operands with fp32 PSUM accumulation: scores^T = K Q^T (row-tiled head
    pairs), exp on ScalarE, diagonal masks on GpSimd, A^T V on PE,
  - after each 512-query block, a 4-wide AllToAll (replica groups
    [0-3],[4-7]) redistributes that quarter's head outputs so core (b,r)
    receives ALL 16 heads for queries 512*q + 128*r .. +128; these four
    collectives overlap with the remaining attention compute,
  - the output projection through the full Wout (fp16) runs per received
    128-query chunk, pipelined behind the collectives.

Host assembles the 8 cores x 4 chunks of [128, 1024] into (2, 2048, 1024).

Projection matmuls run in float32r (TF32-like, ~1e-3 rel err); everything
downstream of the projections is fp16 (≥10-bit mantissa, same PE throughput,
half the SBUF/DMA/collective bytes).
"""

import sys

for _p in ("/opt/trn_rl_repo", "/opt/pypackages"):
    if _p not in sys.path:
        sys.path.insert(0, _p)

import numpy as np

import concourse.bass as bass
import concourse.mybir as mybir
import concourse.tile as tile
from concourse import bacc
from concourse.bass_utils import run_bass_kernel_spmd

B = 2
S = 2048
D = 1024
H = 16
DH = 64
NCORES = 8
SB = 512           # q block (matmul moving dim)
KC = 128           # k chunk (contraction tile)
NSB = S // SB      # 4 q-blocks
NKC = S // KC      # 16 k-chunks
NDC = D // KC      # 8 contraction chunks for the projections

_compiled = None


def _build():
    f32 = mybir.dt.float32
    f16 = mybir.dt.float16
    fr = mybir.dt.float32r
    nc = bacc.Bacc(None, target_bir_lowering=False)

    # host-blocked inputs (own batch / own 4 heads only), fp16
    xt = nc.declare_dram_parameter("xt", [NSB, NDC, KC, SB], f16, isOutput=False)
    wqk = nc.declare_dram_parameter("wqk", [NDC, KC, 4 * KC], f16, isOutput=False)
    wv = nc.declare_dram_parameter("wv", [NDC, KC, 2 * KC], f16, isOutput=False)
    wout = nc.declare_dram_parameter("wout", [2, NDC, KC, SB], f16, isOutput=False)
    bqk = nc.declare_dram_parameter("bqk", [KC, 4], f32, isOutput=False)
    bv = nc.declare_dram_parameter("bv", [1, 2 * KC], f32, isOutput=False)
    bo = nc.declare_dram_parameter("bo", [1, D], f32, isOutput=False)
    vones = nc.declare_dram_parameter("vones", [KC, 16], f16, isOutput=False)
    out_ext = nc.declare_dram_parameter("out", [NSB, KC, D], f32, isOutput=True)

    # AllToAll staging: block for dest t = this core's [128 head dims, 128
    # queries] slab for query sub-chunk 512*q + 128*(t%4).  The exchange is
    # logically within each batch group of 4 cores, but the collective stack
    # only supports 8-wide AllToAll (mesh), so both group halves carry the
    # same data and each receiver dynamically slices its own group's 4
    # sender blocks.  Quarters 0-2 exchange both head pairs in one
    # collective; the tail quarter 3 is split per head pair so its first
    # half-collective overlaps the second head-pair's attention.
    # NOTE: the NEFF-init collective barrier cost scales with the number of
    # collective ops, so keep the count low (5 here).
    a2a_in = [nc.dram_tensor(f"a2a_in{q}", [NCORES, 2, KC, KC], f16)
              for q in range(3)]
    a2a_out = [nc.dram_tensor(f"a2a_out{q}", [NCORES, 2, KC, KC], f16)
               for q in range(3)]
    a2a_in3 = [nc.dram_tensor(f"a2a_in3_{hp}", [NCORES, KC, KC], f16)
               for hp in range(2)]
    a2a_out3 = [nc.dram_tensor(f"a2a_out3_{hp}", [NCORES, KC, KC], f16)
                for hp in range(2)]
    groups = [[0, 1, 2, 3, 4, 5, 6, 7]]

    with tile.TileContext(nc) as tc:
        with (
            tc.tile_pool(name="misc", bufs=1) as mp,
            tc.tile_pool(name="weights", bufs=1) as wp,
            tc.tile_pool(name="xbuf", bufs=1) as xp,
            tc.tile_pool(name="qkv", bufs=1) as qkvp,
            tc.tile_pool(name="pbuf", bufs=1) as pb,
            tc.tile_pool(name="obuf", bufs=1) as op,
            tc.tile_pool(name="evict", bufs=1) as ep,
            tc.tile_pool(name="psum", bufs=1, space="PSUM") as pp,
        ):
            # ---- PE warmup while the initial DMAs land --------------------
            wdum = mp.tile([KC, KC], f16, tag="wdum")
            nc.vector.memset(wdum[:], 0.0)
            for i in range(64):
                psd = pp.tile([KC, SB], f32, tag="accum", bufs=2)
                nc.tensor.matmul(psd[:, 0:KC], wdum[:], wdum[:],
                                 start=True, stop=True)

            # ---- big loads first: unblock the first projection ASAP -------
            # wqk_t cols: k*512 + m*128, m in {Q01, Q23, K01, K23}; first
            # halves (k 0:4) land first so proj(0) can begin sooner
            wqk_t = wp.tile([KC, NDC * 4 * KC], f16, tag="wqk")
            xts = []
            for s in range(NSB):
                xts.append(xp.tile([KC, NDC * SB], f16, tag=f"xt{s}",
                                   name=f"xt{s}"))
            HD = NDC // 2
            for h in range(2):
                nc.gpsimd.dma_start(
                    out=wqk_t[:, h * HD * 4 * KC:(h + 1) * HD * 4 * KC]
                    .rearrange("p (k c) -> p k c", k=HD),
                    in_=wqk[h * HD:(h + 1) * HD].rearrange("k p c -> p k c"),
                )
                nc.gpsimd.dma_start(
                    out=xts[0][:, h * HD * SB:(h + 1) * HD * SB]
                    .rearrange("p (k c) -> p k c", k=HD),
                    in_=xt[0, h * HD:(h + 1) * HD].rearrange("k p c -> p k c"),
                )
            wv_t = wp.tile([KC, NDC * 2 * KC], f16, tag="wv")
            nc.gpsimd.dma_start(
                out=wv_t[:].rearrange("p (k c) -> p k c", k=NDC),
                in_=wv[:].rearrange("k p c -> p k c"),
            )
            nc.gpsimd.dma_start(
                out=xts[1][:].rearrange("p (k c) -> p k c", k=NDC),
                in_=xt[1].rearrange("k p c -> p k c"),
            )

            # ---- small constants -----------------------------------------
            bqk_t = mp.tile([KC, 4], f32, tag="bqk")
            nc.sync.dma_start(out=bqk_t[:], in_=bqk[:])
            bv_row = mp.tile([1, 2 * KC], f32, tag="bv_row")
            nc.sync.dma_start(out=bv_row[:], in_=bv[:])
            bv_bc = mp.tile([KC, 2 * KC], f32, tag="bv_bc")
            nc.gpsimd.partition_broadcast(out_ap=bv_bc[:], in_ap=bv_row[:])
            bo_row = mp.tile([1, D], f32, tag="bo_row")
            nc.sync.dma_start(out=bo_row[:], in_=bo[:])
            bo_bc = mp.tile([KC, D], f32, tag="bo_bc")
            nc.gpsimd.partition_broadcast(out_ap=bo_bc[:], in_ap=bo_row[:])
            vones_sb = mp.tile([KC, 16], f16, tag="vones_sb")
            nc.sync.dma_start(out=vones_sb[:], in_=vones[:])

            # ---- persistent activations ----------------------------------
            # QQ[hp][s]: rows 0:64 = Q^T head 4r+2hp, 64:128 = head 4r+2hp+1
            QQ = [[qkvp.tile([KC, SB], f16, tag=f"QQ{hp}_{s}",
                             name=f"QQ{hp}_{s}") for s in range(NSB)]
                  for hp in range(2)]
            KK = [[qkvp.tile([KC, SB], f16, tag=f"KK{hp}_{s}",
                             name=f"KK{hp}_{s}") for s in range(NSB)]
                  for hp in range(2)]
            # V_all[s]: [128, 4 heads * 4 sc * 65]; head v block at v*260,
            # chunk sc at v*260 + sc*65, col 64 of each chunk = 1.0
            NCS = SB // KC
            V_all = [qkvp.tile([KC, 4 * NCS * (DH + 1)], f16, tag=f"V{s}",
                               name=f"V{s}") for s in range(NSB)]
            for s in range(NSB):
                vv = V_all[s][:].rearrange("p (v c) -> p v c", c=DH + 1)
                nc.vector.tensor_copy(vv[:, :, DH], vones_sb[:])
            # O[hp]: rows 0:64 = head 4r+2hp out^T (normalized), 64:128 =
            # head 4r+2hp+1
            O = [op.tile([KC, S], f16, tag=f"O{hp}", name=f"O{hp}")
                 for hp in range(2)]
            # wout (fp16, 2MB) loaded whole; needed from first out-proj on
            wout_t = wp.tile([KC, 2 * NDC * SB], f16, tag="wout")

            def proj(s):
                """QKV projection for seq block s (f32r)."""
                xs = xts[s]
                for m in range(4):
                    ps = pp.tile([KC, SB], f32, tag="accum", bufs=2,
                                 name=f"psqk{m}_{s}")
                    for k in range(NDC):
                        nc.tensor.matmul(
                            ps[:],
                            wqk_t[:, k * 4 * KC + m * KC:
                                  k * 4 * KC + (m + 1) * KC],
                            xs[:, k * SB:(k + 1) * SB],
                            start=(k == 0),
                            stop=(k == NDC - 1),
                        )
                        if s == 0 and m == 0 and k == 3:
                            # early wout kick: overlaps with projections
                            nc.gpsimd.dma_start(
                                out=wout_t[:].rearrange(
                                    "p (k c) -> p k c", k=2 * NDC),
                                in_=wout[:].rearrange(
                                    "n k p c -> p (n k) c"),
                            )
                    dest = (QQ if m < 2 else KK)[m % 2][s]
                    nc.vector.tensor_scalar_add(
                        dest[:], ps[:], bqk_t[:, m:m + 1])
                for sc in range(NCS):
                    pv = pp.tile([KC, 2 * KC], f32, tag="accum", bufs=2,
                                 name=f"psv{sc}_{s}")
                    for k in range(NDC):
                        nc.tensor.matmul(
                            pv[:],
                            xs[:, k * SB + sc * KC:k * SB + (sc + 1) * KC],
                            wv_t[:, k * 2 * KC:(k + 1) * 2 * KC],
                            start=(k == 0),
                            stop=(k == NDC - 1),
                        )
                    vv = V_all[s][:].rearrange(
                        "p (v k c) -> p v k c", v=4, k=NCS)
                    nc.vector.tensor_add(
                        vv[:, :, sc, 0:DH],
                        pv[:].rearrange("p (v c) -> p v c", c=DH),
                        bv_bc[:].rearrange("p (v c) -> p v c", c=DH),
                    )

            def attn(q, hp):
                """Attention for query block q, one head pair."""
                nkc = 4 * (q + 1)
                if True:
                    P = []
                    for kc in range(nkc):
                        d = kc - 4 * q
                        c0 = KC * max(d, 0)
                        ps = pp.tile([KC, 2 * SB], f32, tag="pss", bufs=2,
                                     name=f"pss{hp}_{q}_{kc}")
                        for hh in range(2):
                            r0 = hh * DH
                            nc.tensor.matmul(
                                ps[:, hh * SB + c0:(hh + 1) * SB],
                                KK[hp][kc // 4][r0:r0 + DH,
                                                (kc % 4) * KC:
                                                (kc % 4 + 1) * KC],
                                QQ[hp][q][r0:r0 + DH, c0:SB],
                                start=True,
                                stop=True,
                            )
                        pt = pb.tile([KC, 2 * SB], f16, tag="P", bufs=8,
                                     name=f"P{hp}_{q}_{kc}")
                        P.append(pt)
                        ps3 = ps[:].rearrange("p (h f) -> p h f", h=2)
                        pd3 = pt[:].rearrange("p (h f) -> p h f", h=2)
                        nc.scalar.activation(
                            pd3[:, :, c0:SB],
                            ps3[:, :, c0:SB],
                            mybir.ActivationFunctionType.Exp,
                            scale=1.0 / float(np.sqrt(DH)),
                        )
                        if d >= 0:  # diagonal chunk: zero where k > q
                            nc.gpsimd.affine_select(
                                out=pd3[:, :, c0:SB],
                                in_=pd3[:, :, c0:SB],
                                pattern=[[0, 2], [1, SB - c0]],
                                compare_op=mybir.AluOpType.is_ge,
                                fill=0.0,
                                base=0,
                                channel_multiplier=-1,
                            )
                    pos = [pp.tile([DH + 1, SB], f32, tag="pos", bufs=2,
                                   name=f"pos{hh}_{hp}_{q}")
                           for hh in range(2)]
                    for kc in range(nkc):
                        d = kc - 4 * q
                        c0 = KC * max(d, 0)
                        for hh in range(2):
                            v = 2 * hp + hh
                            nc.tensor.matmul(
                                pos[hh][:, c0:SB],
                                V_all[kc // 4][:, v * NCS * (DH + 1) +
                                               (kc % 4) * (DH + 1):
                                               v * NCS * (DH + 1) +
                                               (kc % 4 + 1) * (DH + 1)],
                                P[kc][:, hh * SB + c0:(hh + 1) * SB],
                                start=(kc == 0),
                                stop=(kc == nkc - 1),
                            )
                    # normalize: O = pos[0:64] * (1 / pos[64]) per head
                    for hh in range(2):
                        den0 = ep.tile([1, SB], f32, tag=f"den{hh}", bufs=2,
                                       name=f"den{hh}_{hp}_{q}")
                        nc.vector.tensor_copy(den0[:], pos[hh][DH:DH + 1, :])
                        rden = ep.tile([1, SB], f32, tag=f"rden{hh}", bufs=2)
                        rscr = ep.tile([1, SB], f32, tag=f"rscr{hh}", bufs=2)
                        nc.vector.reciprocal_approx_accurate(
                            rden[:], den0[:], rscr[:])
                        rbc = ep.tile([DH, SB], f32, tag=f"rbc{hh}", bufs=2,
                                      name=f"rbc{hh}_{hp}_{q}")
                        nc.gpsimd.partition_broadcast(
                            out_ap=rbc[:],
                            in_ap=rden[:],
                        )
                        r0 = hh * DH
                        nc.vector.tensor_mul(
                            O[hp][r0:r0 + DH, q * SB:(q + 1) * SB],
                            pos[hh][0:DH, :],
                            rbc[:],
                        )
                    # stage this head-pair's quarter (both group halves get
                    # a copy); quarter 3 fires a per-head-pair collective
                    # immediately, quarters 0-2 fire one collective per
                    # quarter after the second head pair
                    osrc = O[hp][:, q * SB:(q + 1) * SB].rearrange(
                        "p (t c) -> p t c", t=4)
                    for g in range(2):
                        if q == 3:
                            dst = a2a_in3[hp][4 * g:4 * (g + 1)].rearrange(
                                "t p c -> p t c")
                        else:
                            dst = a2a_in[q][4 * g:4 * (g + 1), hp].rearrange(
                                "t p c -> p t c")
                        nc.gpsimd.dma_start(out=dst, in_=osrc)
                    if q == 3:
                        nc.gpsimd.collective_compute(
                            "AllToAll",
                            mybir.AluOpType.bypass,
                            replica_groups=groups,
                            ins=[a2a_in3[hp][:]],
                            outs=[a2a_out3[hp][:]],
                        )
                    elif hp == 1:
                        nc.gpsimd.collective_compute(
                            "AllToAll",
                            mybir.AluOpType.bypass,
                            replica_groups=groups,
                            ins=[a2a_in[q][:]],
                            outs=[a2a_out[q][:]],
                        )

            # my group's sender blocks start at slot 4*(rank//4)
            pid = nc.sync.partition_id()
            soff = (pid // 4) * 4

            po_live = {}

            def outproj(q):
                """Output projection for quarter q (whole, quarters 0-2)."""
                recv = ep.tile([KC, NDC * KC], f16, tag="recv", bufs=2,
                               name=f"recv{q}")
                nc.sync.dma_start(
                    out=recv[:].rearrange("p (k c) -> p k c", k=NDC),
                    in_=a2a_out[q][bass.ds(soff, 4)].rearrange(
                        "s h p c -> p (s h) c"),
                )
                for nb in range(2):
                    po = pp.tile([KC, SB], f32, tag="accum", bufs=2,
                                 name=f"po{nb}_{q}")
                    for k in range(NDC):
                        nc.tensor.matmul(
                            po[:],
                            recv[:, k * KC:(k + 1) * KC],
                            wout_t[:, (nb * NDC + k) * SB:
                                   (nb * NDC + k + 1) * SB],
                            start=(k == 0),
                            stop=(k == NDC - 1),
                        )
                    ot = ep.tile([KC, SB], f32, tag="osb", bufs=2,
                                 name=f"ot{nb}_{q}")
                    nc.vector.tensor_add(
                        ot[:], po[:], bo_bc[:, nb * SB:(nb + 1) * SB])
                    nc.sync.dma_start(
                        out=out_ext[q, :, nb * SB:(nb + 1) * SB],
                        in_=ot[:],
                    )

            def outproj3(hp):
                """Half the output projection for quarter 3: contraction
                chunks belonging to head-pair hp (recv cols k = 2*s + hp)."""
                q = 3
                recv = po_live.setdefault(
                    "recv3",
                    ep.tile([KC, NDC * KC], f16, tag="recv", bufs=2,
                            name="recv3"))
                nc.sync.dma_start(
                    out=recv[:].rearrange("p (s h c) -> p s h c",
                                          h=2, c=KC)[:, :, hp],
                    in_=a2a_out3[hp][bass.ds(soff, 4)].rearrange(
                        "s p c -> p s c"),
                )
                for nb in range(2):
                    if hp == 0:
                        po_live[(q, nb)] = pp.tile(
                            [KC, SB], f32, tag="accum", bufs=2,
                            name=f"po{nb}_{q}")
                    po = po_live[(q, nb)]
                    for s in range(4):
                        k = 2 * s + hp
                        nc.tensor.matmul(
                            po[:],
                            recv[:, k * KC:(k + 1) * KC],
                            wout_t[:, (nb * NDC + k) * SB:
                                   (nb * NDC + k + 1) * SB],
                            start=(k == 0),
                            stop=(k == NDC - 1),
                        )
                    if hp == 1:
                        ot = ep.tile([KC, SB], f32, tag="osb", bufs=2,
                                     name=f"ot{nb}_{q}")
                        nc.vector.tensor_add(
                            ot[:], po[:], bo_bc[:, nb * SB:(nb + 1) * SB])
                        nc.sync.dma_start(
                            out=out_ext[q, :, nb * SB:(nb + 1) * SB],
                            in_=ot[:],
                        )

            # ---- pipeline -------------------------------------------------
            proj(0)
            nc.gpsimd.dma_start(
                out=xts[2][:].rearrange("p (k c) -> p k c", k=NDC),
                in_=xt[2].rearrange("k p c -> p k c"),
            )
            proj(1)
            nc.gpsimd.dma_start(
                out=xts[3][:].rearrange("p (k c) -> p k c", k=NDC),
                in_=xt[3].rearrange("k p c -> p k c"),
            )
            attn(0, 0)
            attn(0, 1)
            proj(2)
            attn(1, 0)
            attn(1, 1)
            proj(3)
            attn(2, 0)
            outproj(0)
            attn(2, 1)
            outproj(1)
            attn(3, 0)
            outproj(2)
            attn(3, 1)
            outproj3(0)
            outproj3(1)

    nc.compile()
    return nc


def _get_program():
    global _compiled
    if _compiled is None:
        _compiled = _build()
    return _compiled


def _shard_inputs(x, Wqkv, bqkv, Wout, bout):
    """Build the 8 per-core input maps (all host-side numpy)."""
    x = np.ascontiguousarray(x, dtype=np.float32)
    Wqkv = np.asarray(Wqkv, dtype=np.float32)
    bqkv = np.asarray(bqkv, dtype=np.float32)
    Wout = np.asarray(Wout, dtype=np.float32)
    bout = np.ascontiguousarray(np.asarray(bout, dtype=np.float32))

    Wq = Wqkv[:, 0 * D:1 * D]
    Wk = Wqkv[:, 1 * D:2 * D]
    Wv_full = Wqkv[:, 2 * D:3 * D]
    bq = bqkv[0 * D:1 * D]
    bk = bqkv[1 * D:2 * D]
    bv_full = bqkv[2 * D:3 * D]

    # per batch: [NSB, NDC, KC, SB] blocked transpose of x (fp16)
    xts = []
    for b in range(B):
        xts.append(np.ascontiguousarray(
            x[b].T                                   # [D, S]
            .reshape(NDC, KC, NSB, SB).transpose(2, 0, 1, 3)
            .astype(np.float16)
        ))
    wout_b = np.ascontiguousarray(
        Wout.reshape(NDC, KC, 2, SB).transpose(2, 0, 1, 3)
        .astype(np.float16))
    bo_row = np.ascontiguousarray(bout.reshape(1, D))
    vones = np.ones((KC, 16), dtype=np.float16)

    in_maps = []
    for c in range(NCORES):
        b, r = c // 4, c % 4
        h0 = 4 * r
        cols = lambda W, i: W[:, (h0 + i) * DH:(h0 + i + 2) * DH]
        wqk_c = np.ascontiguousarray(np.concatenate(
            [cols(Wq, 0), cols(Wq, 2), cols(Wk, 0), cols(Wk, 2)],
            axis=1).reshape(NDC, KC, 4 * KC).astype(np.float16))
        bqk_c = np.ascontiguousarray(np.stack(
            [bq[(h0) * DH:(h0 + 2) * DH], bq[(h0 + 2) * DH:(h0 + 4) * DH],
             bk[(h0) * DH:(h0 + 2) * DH], bk[(h0 + 2) * DH:(h0 + 4) * DH]],
            axis=1))
        wv_c = np.ascontiguousarray(
            Wv_full[:, h0 * DH:(h0 + 4) * DH].reshape(NDC, KC, 2 * KC)
            .astype(np.float16))
        bv_c = np.ascontiguousarray(
            bv_full[h0 * DH:(h0 + 4) * DH].reshape(1, 2 * KC))
        in_maps.append({
            "xt": xts[b], "wqk": wqk_c, "wv": wv_c, "wout": wout_b,
            "bqk": bqk_c, "bv": bv_c, "bo": bo_row, "vones": vones,
        })
    return in_maps


def run(inputs, trace=False, trace_kwargs=None):
    nc = _get_program()
    in_maps = _shard_inputs(**inputs)
    res = run_bass_kernel_spmd(
        nc, in_maps, list(range(NCORES)), trace=trace,
        **(trace_kwargs or {}),
    )
    out = np.empty((B, S, D), dtype=np.float32)
    for c in range(NCORES):
        b, r = c // 4, c % 4
        o = res.results[c]["out"]          # [NSB, KC, D]
        for q in range(NSB):
            out[b, SB * q + KC * r:SB * q + KC * (r + 1), :] = o[q]
    return out, res


def kernel(**inputs):
    out, _ = run(inputs)
    return out
